# revision 8
# baseline (speedup 1.0000x reference)
"""Trainium2 kernel for DSN (deep subspace networks) few-shot classification.

Math: reference computes, per class w with orthonormal subspace basis U_w
([d, k]), dist_w(q) = ||q - U_w U_w^T q||^2 = ||q||^2 - ||U_w^T q||^2 and
returns log_softmax(-dist) over classes. The -||q||^2 term is constant per
row, so log_softmax(-dist)[q, :] == log_softmax(s)[q, :] with
s[q, w] = ||U_w^T q||^2.

Host (tiny): group support by class, SVD -> U_w, stack W = [U_0 .. U_4]
([1024, 45]), cast to fp16, pack the query shard into the exact SBUF
layout so every block DMA is 128 x 8KB contiguous. Rowwise log_softmax on
the [16384, 5] score matrix also runs on host (82K elements).
Device (memory-bound): stream 4 x 1MB query blocks on the sync HWDGE ring;
per block: 8 accumulating PE matmuls -> ct psum [45, 512], square on DVE
(psum -> sbuf fp16), indicator matmul -> s psum [5, 512], DVE copy into a
[5, 2048] staging tile; one contiguous 40KB output DMA at the end.

Sharding: data-parallel over the 16384 query rows, 2048 per core, SPMD on
8 NeuronCores. No cross-core communication.
"""

import numpy as np

import concourse.bass as bass
import concourse.bacc as bacc
import concourse.mybir as mybir
from concourse.tile import TileContext
from concourse.vector_clock import ScopedClock
from concourse.bass_utils import run_bass_kernel_spmd


class FastTileContext(TileContext):
    """TileContext with a slim kernel tail.

    The stock tail is drain -> all-engine barrier -> semaphore clear ->
    all-engine barrier (~10 us of EVSEM butterflies). The Bass preamble
    already clears the whole bass semaphore range at kernel start, so for
    a one-shot kernel the trailing clear + barriers are redundant; the
    drain (which waits on the global vector clock, i.e. every engine and
    DMA queue) is what guarantees completion.
    """

    def _drain_and_barrier(self, tick_clock, wait_clock):
        drain_inst = self.nc.sync.drain()
        wait_clock.add_sem_waits(
            drain_inst.ins, ScopedClock({None: tick_clock.global_clock})
        )
        popped = self.nc._tile_sem_poison_stack.pop()
        assert popped is self._sem_poison

# Problem geometry (hardcoded per spec).
N_CORES = 8
N_QUERY = 16384
D = 1024
N_WAY = 5
N_SHOT = 10
K = N_SHOT - 1            # 9 basis vectors per class
M = N_WAY * K             # 45 stacked basis columns
NQ = N_QUERY // N_CORES   # 2048 query rows per core
DC = D // 128             # 8 contraction chunks of 128
NB = NQ // 512            # 4 query blocks of 512 per core

FP16 = mybir.dt.float16
FP32 = mybir.dt.float32
ALU = mybir.AluOpType

# Query blocks per core (sizes sum to NQ). Early blocks are big (the DMA
# stream is the bottleneck there); late blocks are small so the
# end-of-stream compute + output-DMA chain is short. The first block's
# DMA lands in quarter pieces so the PE can start ~2us earlier.
BLOCKS = [512, 512, 512, 256, 256]
# Input DMA piece granularity per block, in d-chunks (DC=8 chunks/block).
PIECES = [2, 4, 4, 8, 4]   # chunks per piece: b0 4x, b1/b2 2x, b3 1x, b4 2x

_CACHE = {}


def _build_bass():
    nc = bacc.Bacc("TRN2", target_bir_lowering=False, debug=False,
                   num_devices=N_CORES)
    qt = nc.declare_dram_parameter("qt", [128, NQ * DC], FP16,
                                   isOutput=False)
    wfull = nc.declare_dram_parameter("wfull", [128, DC * M + N_WAY], FP16,
                                      isOutput=False)
    out = nc.declare_dram_parameter("out", [N_WAY, NQ], FP32, isOutput=True)

    with FastTileContext(nc) as tc:
        with (
            tc.tile_pool(name="const", bufs=1) as cpool,
            tc.tile_pool(name="qp", bufs=1) as qpool,
            tc.tile_pool(name="wk", bufs=3) as wk,
            tc.tile_pool(name="ps_ct", bufs=2, space="PSUM") as ps_ct,
            tc.tile_pool(name="ps_s", bufs=2, space="PSUM") as ps_s,
        ):
            wtile = cpool.tile([128, DC * M + N_WAY], FP16)
            nc.sync.dma_start(out=wtile, in_=wfull[:, :])
            ind = wtile[0:M, DC * M:DC * M + N_WAY]      # [45, 5]

            # Prefetch the Square ACT table while the query stream is in
            # flight (first real square would otherwise eat the ~1.3us
            # table load on the block-0 critical path).
            warm = cpool.tile([1, 2], FP32)
            nc.vector.memset(warm[:, 0:1], 0.0)
            nc.scalar.square(warm[:, 1:2], warm[:, 0:1])

            qtiles = [qpool.tile([128, DC, sz], FP16, name=f"qb{b}",
                                 tag=f"q{b}")
                      for b, sz in enumerate(BLOCKS)]
            s_sbuf = qpool.tile([N_WAY, NQ], FP32)

            # Streaming input DMAs, strictly ordered on the sync HWDGE
            # ring so completion semaphores fire block by block.
            off = 0
            for b, sz in enumerate(BLOCKS):
                step = PIECES[b]
                for c0 in range(0, DC, step):
                    n = step * sz
                    nc.sync.dma_start(
                        out=qtiles[b][:, c0:c0 + step, :],
                        in_=qt[:, off + c0 * sz: off + c0 * sz + n].rearrange(
                            "p (c q) -> p c q", c=step),
                    )
                off += DC * sz

            st = 0
            for b, sz in enumerate(BLOCKS):
                ct = ps_ct.tile([M, sz], FP32, tag=f"ct{sz}")
                for c in range(DC):
                    nc.tensor.matmul(
                        ct,
                        lhsT=wtile[:, c * M:(c + 1) * M],
                        rhs=qtiles[b][:, c, :],
                        start=(c == 0),
                        stop=(c == DC - 1),
                    )
                ctsq = wk.tile([M, sz], FP16, tag=f"ctsq{sz}")
                nc.scalar.square(ctsq, ct)

                sps = ps_s.tile([N_WAY, sz], FP32, tag=f"sps{sz}")
                nc.tensor.matmul(sps, lhsT=ind, rhs=ctsq,
                                 start=True, stop=True)
                nc.vector.tensor_scalar_mul(s_sbuf[:, st:st + sz], sps, 1.0)
                # Ship this block's scores on the scalar HWDGE ring so the
                # sync ring keeps streaming query data uninterrupted.
                nc.scalar.dma_start(out=out[:, st:st + sz],
                                    in_=s_sbuf[:, st:st + sz])
                st += sz
    nc.compile()
    return nc


def _host_prep(train_imgs, train_labels, query_imgs):
    """Per-class subspace bases (tiny SVDs) + fp16 device operands."""
    n_support, n_way = train_labels.shape
    n_shot = n_support // n_way
    cls = np.argmax(np.asarray(train_labels), axis=1)
    order = np.argsort(cls, kind="stable")
    grouped = np.asarray(train_imgs, np.float64)[order].reshape(
        n_way, n_shot, -1)
    mats = np.swapaxes(grouped, 1, 2)                    # [w, d, s]
    U, _, _ = np.linalg.svd(mats, full_matrices=False)   # [w, d, s]
    W = np.concatenate([U[w][:, :n_shot - 1] for w in range(n_way)],
                       axis=1)                           # [d, 45]

    # Device layout: wfull[p, c*45 + m] = W[c*128 + p, m]; indicator appended.
    wfull = np.zeros((128, DC * M + N_WAY), np.float16)
    wfull[:, :DC * M] = (
        W.reshape(DC, 128, M).transpose(1, 0, 2).reshape(128, DC * M)
    ).astype(np.float16)
    for w in range(N_WAY):
        wfull[w * K:(w + 1) * K, DC * M + w] = 1.0

    qh = np.asarray(query_imgs, np.float32).astype(np.float16)
    return wfull, qh


def _pack_core(qh, k):
    """Block-major packing matching the SBUF tiles: for each block
    [st, st+sz), dram cols [8*st + c*sz + j] = Q[k*2048 + st + j, c*128 + p].
    """
    qsh = qh[k * NQ:(k + 1) * NQ]                        # [2048, 1024]
    parts = []
    st = 0
    for sz in BLOCKS:
        blk = qsh[st:st + sz].reshape(sz, DC, 128)       # [j, c, p]
        parts.append(blk.transpose(2, 1, 0).reshape(128, DC * sz))
        st += sz
    return np.ascontiguousarray(np.concatenate(parts, axis=1))


def _run(inputs, trace=False, **kwargs):
    if "nc" not in _CACHE:
        _CACHE["nc"] = _build_bass()
    nc = _CACHE["nc"]

    wfull, qh = _host_prep(inputs["train_imgs"], inputs["train_labels"],
                           inputs["query_imgs"])
    in_maps = [{"qt": _pack_core(qh, k), "wfull": wfull}
               for k in range(N_CORES)]

    res = run_bass_kernel_spmd(nc, in_maps, core_ids=list(range(N_CORES)),
                               trace=trace, **kwargs)
    s = np.concatenate([res.results[k]["out"].T for k in range(N_CORES)],
                       axis=0)                           # [16384, 5] fp32
    m = s.max(axis=1, keepdims=True)
    e = np.exp(s - m, dtype=np.float32)
    full = (s - m - np.log(e.sum(axis=1, keepdims=True),
                           dtype=np.float32)).astype(np.float32)
    return full, res


def kernel(**inputs) -> np.ndarray:
    out, _ = _run(inputs)
    return out


# revision 13
# speedup vs baseline: 1.1027x; 1.1027x over previous
"""Trainium2 kernel for DSN (deep subspace networks) few-shot classification.

Math: reference computes, per class w with orthonormal subspace basis U_w
([d, k]), dist_w(q) = ||q - U_w U_w^T q||^2 = ||q||^2 - ||U_w^T q||^2 and
returns log_softmax(-dist) over classes. The -||q||^2 term is constant per
row, so log_softmax(-dist)[q, :] == log_softmax(s)[q, :] with
s[q, w] = ||U_w^T q||^2.

Host (tiny): group support by class, SVD -> U_w, stack W = [U_0 .. U_4]
([1024, 45]), cast to fp16, pack the query shard into the exact SBUF
layout so every block DMA is 128 x 8KB contiguous. Rowwise log_softmax on
the [16384, 5] score matrix also runs on host (82K elements).
Device (memory-bound): stream 4 x 1MB query blocks on the sync HWDGE ring;
per block: 8 accumulating PE matmuls -> ct psum [45, 512], square on DVE
(psum -> sbuf fp16), indicator matmul -> s psum [5, 512], DVE copy into a
[5, 2048] staging tile; one contiguous 40KB output DMA at the end.

Sharding: data-parallel over the 16384 query rows, 2048 per core, SPMD on
8 NeuronCores. No cross-core communication.
"""

import numpy as np

import concourse.bass as bass
import concourse.bacc as bacc
import concourse.mybir as mybir
from concourse.tile import TileContext
from concourse.vector_clock import ScopedClock
from concourse.bass_utils import run_bass_kernel_spmd


class FastTileContext(TileContext):
    """TileContext with a slim kernel tail.

    The stock tail is drain -> all-engine barrier -> semaphore clear ->
    all-engine barrier (~10 us of EVSEM butterflies). The Bass preamble
    already clears the whole bass semaphore range at kernel start, so for
    a one-shot kernel the trailing clear + barriers are redundant; the
    drain (which waits on the global vector clock, i.e. every engine and
    DMA queue) is what guarantees completion.
    """

    def _drain_and_barrier(self, tick_clock, wait_clock):
        drain_inst = self.nc.sync.drain()
        wait_clock.add_sem_waits(
            drain_inst.ins, ScopedClock({None: tick_clock.global_clock})
        )
        popped = self.nc._tile_sem_poison_stack.pop()
        assert popped is self._sem_poison

# Problem geometry (hardcoded per spec).
N_CORES = 8
N_QUERY = 16384
D = 1024
N_WAY = 5
N_SHOT = 10
K = N_SHOT - 1            # 9 basis vectors per class
M = N_WAY * K             # 45 stacked basis columns
NQ = N_QUERY // N_CORES   # 2048 query rows per core
DC = D // 128             # 8 contraction chunks of 128
NB = NQ // 512            # 4 query blocks of 512 per core

FP16 = mybir.dt.float16
FP32 = mybir.dt.float32
ALU = mybir.AluOpType

# Query blocks per core (sizes sum to NQ). Early blocks are big (the DMA
# stream is the bottleneck there); late blocks are small so the
# end-of-stream compute + output-DMA chain is short. The first block's
# DMA lands in two half pieces so the PE can start ~1.5us earlier; all
# other pieces stay large (big descriptors stream at full HBM rate).
BLOCKS = [512, 512, 512, 256, 256]
# Input DMA piece granularity per block, in d-chunks (DC=8 chunks/block).
PIECES = [4, 8, 8, 8, 8]
# PE warm-up matmuls on the weight tile: ~14 x 365 cols at 1.2GHz cold
# ~= 4.2us of sustained PE busy, enough to flip the HAM clock gate to
# 2.4GHz before the real matmuls arrive. They run while the query
# stream is still in flight (PE is idle then anyway).
N_WARMUP_MM = 14

_CACHE = {}


def _build_bass():
    nc = bacc.Bacc("TRN2", target_bir_lowering=False, debug=False,
                   num_devices=N_CORES)
    qt = nc.declare_dram_parameter("qt", [128, NQ * DC], FP16,
                                   isOutput=False)
    wfull = nc.declare_dram_parameter("wfull", [128, DC * M + N_WAY], FP16,
                                      isOutput=False)
    out = nc.declare_dram_parameter("out", [N_WAY, NQ], FP32, isOutput=True)

    with FastTileContext(nc) as tc:
        with (
            tc.tile_pool(name="const", bufs=1) as cpool,
            tc.tile_pool(name="qp", bufs=1) as qpool,
            tc.tile_pool(name="wk", bufs=3) as wk,
            tc.tile_pool(name="ps_ct", bufs=2, space="PSUM") as ps_ct,
            tc.tile_pool(name="ps_s", bufs=2, space="PSUM") as ps_s,
            tc.tile_pool(name="ps_w", bufs=1, space="PSUM") as ps_w,
        ):
            wtile = cpool.tile([128, DC * M + N_WAY], FP16)
            nc.sync.dma_start(out=wtile, in_=wfull[:, :])
            ind = wtile[0:M, DC * M:DC * M + N_WAY]      # [45, 5]

            # Prefetch the Square ACT table while the query stream is in
            # flight (first real square would otherwise eat the ~1.3us
            # table load on the block-0 critical path).
            warm = cpool.tile([1, 2], FP32)
            nc.vector.memset(warm[:, 0:1], 0.0)
            nc.scalar.square(warm[:, 1:2], warm[:, 0:1])

            # HAM warm-up: sustained PE busy on the tiny weight tile.
            ps_warm = ps_w.tile([M, DC * M + N_WAY], FP32)
            for _ in range(N_WARMUP_MM):
                nc.tensor.matmul(ps_warm, lhsT=wtile[:, 0:M], rhs=wtile,
                                 start=True, stop=True)

            qtiles = [qpool.tile([128, DC, sz], FP16, name=f"qb{b}",
                                 tag=f"q{b}")
                      for b, sz in enumerate(BLOCKS)]
            s_sbuf = qpool.tile([N_WAY, NQ], FP32)

            # Streaming input DMAs, strictly ordered on the sync HWDGE
            # ring so completion semaphores fire block by block.
            off = 0
            for b, sz in enumerate(BLOCKS):
                step = PIECES[b]
                for c0 in range(0, DC, step):
                    n = step * sz
                    nc.sync.dma_start(
                        out=qtiles[b][:, c0:c0 + step, :],
                        in_=qt[:, off + c0 * sz: off + c0 * sz + n].rearrange(
                            "p (c q) -> p c q", c=step),
                    )
                off += DC * sz

            st = 0
            for b, sz in enumerate(BLOCKS):
                ctf = ps_ct.tile([M, 512], FP32, tag="ct")
                ct = ctf[:, 0:sz]
                for c in range(DC):
                    nc.tensor.matmul(
                        ct,
                        lhsT=wtile[:, c * M:(c + 1) * M],
                        rhs=qtiles[b][:, c, :],
                        start=(c == 0),
                        stop=(c == DC - 1),
                    )
                ctsqf = wk.tile([M, 512], FP16, tag="ctsq")
                ctsq = ctsqf[:, 0:sz]
                nc.scalar.square(ctsq, ct)

                spsf = ps_s.tile([N_WAY, 512], FP32, tag="sps")
                sps = spsf[:, 0:sz]
                nc.tensor.matmul(sps, lhsT=ind, rhs=ctsq,
                                 start=True, stop=True)
                nc.vector.tensor_scalar_mul(s_sbuf[:, st:st + sz], sps, 1.0)
                # Ship this block's scores on the scalar HWDGE ring so the
                # sync ring keeps streaming query data uninterrupted.
                nc.scalar.dma_start(out=out[:, st:st + sz],
                                    in_=s_sbuf[:, st:st + sz])
                st += sz
    nc.compile()
    return nc


def _host_prep(train_imgs, train_labels, query_imgs):
    """Per-class subspace bases (tiny SVDs) + fp16 device operands."""
    n_support, n_way = train_labels.shape
    n_shot = n_support // n_way
    cls = np.argmax(np.asarray(train_labels), axis=1)
    order = np.argsort(cls, kind="stable")
    grouped = np.asarray(train_imgs, np.float64)[order].reshape(
        n_way, n_shot, -1)
    mats = np.swapaxes(grouped, 1, 2)                    # [w, d, s]
    U, _, _ = np.linalg.svd(mats, full_matrices=False)   # [w, d, s]
    W = np.concatenate([U[w][:, :n_shot - 1] for w in range(n_way)],
                       axis=1)                           # [d, 45]

    # Device layout: wfull[p, c*45 + m] = W[c*128 + p, m]; indicator appended.
    wfull = np.zeros((128, DC * M + N_WAY), np.float16)
    wfull[:, :DC * M] = (
        W.reshape(DC, 128, M).transpose(1, 0, 2).reshape(128, DC * M)
    ).astype(np.float16)
    for w in range(N_WAY):
        wfull[w * K:(w + 1) * K, DC * M + w] = 1.0

    qh = np.asarray(query_imgs, np.float32).astype(np.float16)
    return wfull, qh


def _pack_core(qh, k):
    """Block-major packing matching the SBUF tiles: for each block
    [st, st+sz), dram cols [8*st + c*sz + j] = Q[k*2048 + st + j, c*128 + p].
    """
    qsh = qh[k * NQ:(k + 1) * NQ]                        # [2048, 1024]
    parts = []
    st = 0
    for sz in BLOCKS:
        blk = qsh[st:st + sz].reshape(sz, DC, 128)       # [j, c, p]
        parts.append(blk.transpose(2, 1, 0).reshape(128, DC * sz))
        st += sz
    return np.ascontiguousarray(np.concatenate(parts, axis=1))


def _run(inputs, trace=False, **kwargs):
    if "nc" not in _CACHE:
        _CACHE["nc"] = _build_bass()
    nc = _CACHE["nc"]

    wfull, qh = _host_prep(inputs["train_imgs"], inputs["train_labels"],
                           inputs["query_imgs"])
    in_maps = [{"qt": _pack_core(qh, k), "wfull": wfull}
               for k in range(N_CORES)]

    res = run_bass_kernel_spmd(nc, in_maps, core_ids=list(range(N_CORES)),
                               trace=trace, **kwargs)
    s = np.concatenate([res.results[k]["out"].T for k in range(N_CORES)],
                       axis=0)                           # [16384, 5] fp32
    m = s.max(axis=1, keepdims=True)
    e = np.exp(s - m, dtype=np.float32)
    full = (s - m - np.log(e.sum(axis=1, keepdims=True),
                           dtype=np.float32)).astype(np.float32)
    return full, res


def kernel(**inputs) -> np.ndarray:
    out, _ = _run(inputs)
    return out
